# revision 44
# baseline (speedup 1.0000x reference)
"""Drosophila optic lobe circuit simulation on 8 Trainium2 NeuronCores.

Edge/target-sharded across 8 devices; batch rides partitions.
- N padded 49000->49152 = 8 dev x 8 cores x 768 targets.
- Gather tables: partition p = 16k + 8h + b holds r=relu(v) of batch b,
  source-half h ([24576] fp32). ap_gather per 1600-slot chunk fetches
  r[src] for all 8 batches; wrong-half rows masked by wmask=0.
- currents = gathered * wmask (fp16 static weights streamed from DRAM).
- Scatter-add = carried inclusive cumsum (tensor_tensor_scan) + boundary
  extraction (small ap_gather per chunk) + first difference.
- Source halves folded by a static 0/1 matmul; v updated in extract-column
  layout; r re-sharded to id order (ap_gather), AllGathered across devices,
  tables rebuilt by broadcast DMAs.

Host->device transfer over the axon tunnel is the wall-clock bottleneck
and scales with the LARGEST single parameter, not total bytes (arrays
stream in parallel). So inputs are shipped deduplicated (weights are
batch-invariant: 16 unique rows, not 128; v0 is half-invariant; mdt is
per-core) and split into ~200KB chunks. The r-table (12.6MB, formerly an
input) is built on device by running the publish path once before the
loop. Weights are expanded once into device DRAM (wmx) and streamed
per-chunk from there each step, keeping the steady-state instruction
stream unchanged.
"""

import numpy as np
import sys

sys.path.insert(0, "/opt/trn_rl_repo")

import jax

import concourse.bacc as bacc
import concourse.mybir as mybir
from concourse.tile import TileContext
from concourse.bass_utils import run_bass_kernel_spmd

# Cache XLA executables on disk so repeat dispatches skip the re-compile
# path (the jit closure inside run_bass_via_pjrt is fresh per call, so
# jax's in-memory caches never hit).
jax.config.update("jax_compilation_cache_dir", "/tmp/jax_cache_bass")
jax.config.update("jax_persistent_cache_min_compile_time_secs", 0.0)
jax.config.update("jax_persistent_cache_min_entry_size_bytes", 0)

NREAL = 49000
B = 8
DT = 0.1
NDEV = 8
N = 49152
NH = N // 2
NDEVT = N // NDEV          # 6144
NCORES = 8
MCORE = NDEVT // NCORES    # 768
NCHUNK = 5
CHUNK = 5120
LCORE = NCHUNK * CHUNK     # 25600
BCH = 192
ECOLS = NCHUNK * BCH       # 960

_cache = {}


def _build(steps, with_bias, do_collective=True, do_publish=True, do_chunks=True,
           ag_shared=True, use_for_i=False, gprobe="none", agbufs=None):
    # use_for_i: tc.For_i hardware loops execute, but collective_compute
    # inside the loop body fails at runtime on this stack — keep unrolled.
    # gprobe: timing-only ap_gather scaling probes ("elems"/"idxs"), wrong numerics.
    nc = bacc.Bacc(None)
    f32, f16, i16 = mybir.dt.float32, mybir.dt.float16, mybir.dt.int16

    # compact, split inputs (transfer wall ~ largest single param)
    wm_in = [nc.declare_dram_parameter(f"wm{j}", [16, LCORE // 32], f16,
                                       isOutput=False) for j in range(32)]
    ix_in = [nc.declare_dram_parameter(f"ix{c}", [128, LCORE // 256], i16,
                                       isOutput=False) for c in range(16)]
    v0_in = [nc.declare_dram_parameter(f"v0{j}", [8, ECOLS // 2], f32,
                                       isOutput=False) for j in range(16)]
    fold_in = [nc.declare_dram_parameter(f"fold{j}", [32, 128], f32,
                                         isOutput=False) for j in range(4)]
    mdt_in = [nc.declare_dram_parameter(f"mdt{j}", [8, ECOLS // 2], f32,
                                        isOutput=False) for j in range(2)]
    if with_bias:
        bm_in = [nc.declare_dram_parameter(f"bm{j}", [8, ECOLS // 2], f32,
                                           isOutput=False) for j in range(2)]
    bidx_in = nc.declare_dram_parameter("bidx", [128, ECOLS // 16], i16, isOutput=False)
    idx3_in = nc.declare_dram_parameter("idx3", [128, MCORE // 16], i16, isOutput=False)
    vout = nc.declare_dram_parameter("vout", [B, NDEVT], f16, isOutput=True)

    with TileContext(nc) as tc:
        with (
            tc.tile_pool(name="big", bufs=1) as big,
            tc.tile_pool(name="gbuf", bufs=1) as gbuf,
            tc.tile_pool(name="wbuf", bufs=2) as wbuf,
            tc.tile_pool(name="cs", bufs=2) as csp,
            tc.tile_pool(name="small", bufs=1) as small,
            tc.tile_pool(name="psum", bufs=1, space="PSUM") as psum,
            tc.tile_pool(name="dram", bufs=1, space="DRAM") as dram,
            tc.tile_pool(name="agpool",
                         bufs=agbufs or (2 if use_for_i else max(steps, 1)),
                         space="DRAM") as agp,
        ):
            tbl = big.tile([128, NH], f32, tag="tbl")
            idx1 = small.tile([128, LCORE // 16], i16, tag="idx1")
            bidx = small.tile([128, ECOLS // 16], i16, tag="bidx")
            idx3 = small.tile([128, MCORE // 16], i16, tag="idx3")
            v = small.tile([128, ECOLS], f32, tag="v")
            mdt = small.tile([128, ECOLS], f32, tag="mdt")
            fold = small.tile([128, 128], f32, tag="fold")
            bm = small.tile([128, ECOLS], f32, tag="bm") if with_bias else None
            E = small.tile([128, ECOLS], f32, tag="E")
            syn = small.tile([128, ECOLS], f32, tag="syn")
            rslab = small.tile([128, MCORE], f32, tag="rslab")

            slab_d = dram.tile([B, NDEVT], f32)
            wmx = dram.tile([128, LCORE], f16)

            # ---- input loads / on-device expansion (one-time) ----
            for j in range(16):
                nc.sync.dma_start(out=idx1[:, j * 100:(j + 1) * 100], in_=ix_in[j][:])
            nc.sync.dma_start(out=bidx[:], in_=bidx_in[:])
            nc.sync.dma_start(out=idx3[:], in_=idx3_in[:])
            for j in range(4):
                nc.sync.dma_start(out=fold[32 * j:32 * (j + 1), :], in_=fold_in[j][:])
            HC = ECOLS // 2
            mdtv = mdt[:].rearrange("(k r) n -> k r n", k=8)
            for r in range(16):
                nc.sync.dma_start(out=mdtv[:, r, 0:HC], in_=mdt_in[0][:])
                nc.sync.dma_start(out=mdtv[:, r, HC:ECOLS], in_=mdt_in[1][:])
            if with_bias:
                bmv = bm[:].rearrange("(k r) n -> k r n", k=8)
                for r in range(16):
                    nc.sync.dma_start(out=bmv[:, r, 0:HC], in_=bm_in[0][:])
                    nc.sync.dma_start(out=bmv[:, r, HC:ECOLS], in_=bm_in[1][:])
            vv = v[:].rearrange("(k h b) n -> k h b n", k=8, h=2)
            for h in range(2):
                for b in range(B):
                    nc.sync.dma_start(out=vv[:, h, b, 0:HC], in_=v0_in[2 * b][:])
                    nc.sync.dma_start(out=vv[:, h, b, HC:ECOLS], in_=v0_in[2 * b + 1][:])
            # expand weights (batch-invariant rows) into DRAM wmx once,
            # in param-width units (chunk-size agnostic)
            WQ = LCORE // 32          # 800 cols per wm param
            for t in range(16):
                w = wbuf.tile([128, 2 * WQ], f16, tag="wst")
                wv = w[:].rearrange("(g b) n -> g b n", g=16)
                for b in range(B):
                    nc.sync.dma_start(out=wv[:, b, 0:WQ], in_=wm_in[2 * t][:])
                    nc.sync.dma_start(out=wv[:, b, WQ:2 * WQ], in_=wm_in[2 * t + 1][:])
                nc.sync.dma_start(out=wmx[:, t * 2 * WQ:(t + 1) * 2 * WQ], in_=w[:])

            def publish():
                # r = relu(v) -> id-order slab -> DRAM -> AllGather(f32) ->
                # 16 broadcast DMAs rebuild the replicated table. Fewer
                # instructions beats the PE-matmul rebuild: per-instruction
                # dispatch overhead (~25us) dominates at this size.
                nc.scalar.activation(syn[:], v[:], mybir.ActivationFunctionType.Relu)
                nc.gpsimd.ap_gather(out_ap=rslab[:], in_ap=syn[:], idxs_ap=idx3[:],
                                    channels=128, num_elems=ECOLS, d=1, num_idxs=MCORE)
                for k in range(NCORES):
                    eng = nc.sync if k % 2 == 0 else nc.scalar
                    eng.dma_start(out=slab_d[:, k * MCORE:(k + 1) * MCORE],
                                  in_=rslab[16 * k:16 * k + 8, :])
                if ag_shared:
                    ag_d = agp.tile([NDEV * B, NDEVT], f32, addr_space="Shared", tag="ag")
                else:
                    ag_d = agp.tile([NDEV * B, NDEVT], f32, tag="ag")
                if do_collective:
                    nc.gpsimd.collective_compute(
                        "AllGather", mybir.AluOpType.bypass,
                        replica_groups=[list(range(NDEV))],
                        ins=[slab_d[:]], outs=[ag_d[:]],
                    )
                agv = ag_d[:].rearrange("(d b) n -> d b n", d=NDEV)
                for h in range(2):
                    for b in range(B):
                        eng = nc.sync if b % 2 == 0 else nc.scalar
                        eng.dma_start(
                            out=tbl[:].rearrange("(k r) n -> k r n", k=8)[:, 8 * h + b, :],
                            in_=agv[4 * h:4 * h + 4, b, :][None]
                                .to_broadcast([8, 4, NDEVT]),
                        )

            def step_body(last):
                if not do_chunks:
                    if not last and do_publish:
                        publish()
                    return
                s_tiles = {}
                for c in range(NCHUNK):
                    g = gbuf.tile([128, CHUNK], f32, tag="g")
                    w = wbuf.tile([128, CHUNK], f16, tag="w")
                    s = csp.tile([128, CHUNK], f32, tag="s")
                    s_tiles[c] = s
                    nc.sync.dma_start(out=w[:], in_=wmx[:, c * CHUNK:(c + 1) * CHUNK])
                    if gprobe == "elems":
                        # timing probe: tiny table, same idx count
                        nc.gpsimd.ap_gather(
                            out_ap=g[:], in_ap=tbl[:, 0:1600],
                            idxs_ap=idx1[:, c * (CHUNK // 16):(c + 1) * (CHUNK // 16)],
                            channels=128, num_elems=1600, d=1, num_idxs=CHUNK,
                        )
                    elif gprobe == "idxs":
                        # timing probe: quarter idx count, full table
                        nc.gpsimd.ap_gather(
                            out_ap=g[:, 0:400], in_ap=tbl[:],
                            idxs_ap=idx1[:, c * (CHUNK // 16):c * (CHUNK // 16) + 25],
                            channels=128, num_elems=NH, d=1, num_idxs=400,
                        )
                    else:
                        nc.gpsimd.ap_gather(
                            out_ap=g[:], in_ap=tbl[:],
                            idxs_ap=idx1[:, c * (CHUNK // 16):(c + 1) * (CHUNK // 16)],
                            channels=128, num_elems=NH, d=1, num_idxs=CHUNK,
                        )
                    # extract previous chunk's boundaries (after this gather so
                    # POOL doesn't stall on the DVE scan)
                    sp = s_tiles.pop(c - 1) if c >= 1 else None
                    if c >= 1:
                        nc.gpsimd.ap_gather(
                            out_ap=E[:, (c - 1) * BCH:c * BCH], in_ap=sp[:],
                            idxs_ap=bidx[:, (c - 1) * (BCH // 16):c * (BCH // 16)],
                            channels=128, num_elems=CHUNK, d=1, num_idxs=BCH,
                        )
                    nc.vector.tensor_tensor(out=g[:], in0=g[:], in1=w[:],
                                            op=mybir.AluOpType.mult)
                    init = 0.0 if c == 0 else sp[:, CHUNK - 1:CHUNK]
                    nc.vector.tensor_tensor_scan(
                        out=s[:], data0=g[:], data1=g[:], initial=init,
                        op0=mybir.AluOpType.add, op1=mybir.AluOpType.bypass,
                    )
                c = NCHUNK
                sp = s_tiles.pop(c - 1)
                nc.gpsimd.ap_gather(
                    out_ap=E[:, (c - 1) * BCH:c * BCH], in_ap=sp[:],
                    idxs_ap=bidx[:, (c - 1) * (BCH // 16):c * (BCH // 16)],
                    channels=128, num_elems=CHUNK, d=1, num_idxs=BCH,
                )
                # fold halves: Ef = fold.T @ E
                ef = psum.tile([128, ECOLS], f32, tag="ef")
                for j in range(0, ECOLS, 512):
                    jw = min(512, ECOLS - j)
                    nc.tensor.matmul(out=ef[:, j:j + jw], lhsT=fold[:],
                                     rhs=E[:, j:j + jw], start=True, stop=True)
                # segment sums by first difference (via SBUF copy of ef)
                nc.vector.tensor_copy(out=syn[:], in_=ef[:])
                nc.vector.tensor_copy(out=E[:, 0:1], in_=syn[:, 0:1])
                nc.vector.tensor_tensor(out=E[:, 1:ECOLS], in0=syn[:, 1:ECOLS],
                                        in1=syn[:, 0:ECOLS - 1],
                                        op=mybir.AluOpType.subtract)
                # v += mdt * (E - v) (+ bm)
                nc.vector.tensor_tensor(out=E[:], in0=E[:], in1=v[:],
                                        op=mybir.AluOpType.subtract)
                nc.vector.tensor_tensor(out=E[:], in0=E[:], in1=mdt[:],
                                        op=mybir.AluOpType.mult)
                nc.vector.tensor_tensor(out=v[:], in0=v[:], in1=E[:],
                                        op=mybir.AluOpType.add)
                if with_bias:
                    nc.vector.tensor_tensor(out=v[:], in0=v[:], in1=bm[:],
                                            op=mybir.AluOpType.add)
                if not last and do_publish:
                    publish()

            if steps >= 1:
                publish()
                if use_for_i:
                    if steps > 1:
                        with tc.For_i(0, steps - 1):
                            step_body(last=False)
                    step_body(last=True)
                else:
                    for s in range(steps):
                        step_body(last=(s == steps - 1))

            nc.gpsimd.ap_gather(out_ap=rslab[:], in_ap=v[:], idxs_ap=idx3[:],
                                channels=128, num_elems=ECOLS, d=1, num_idxs=MCORE)
            vslab16 = small.tile([128, MCORE], f16, tag="vslab16")
            nc.vector.tensor_copy(out=vslab16[:], in_=rslab[:])
            for k in range(NCORES):
                nc.sync.dma_start(out=vout[:, k * MCORE:(k + 1) * MCORE],
                                  in_=vslab16[16 * k:16 * k + 8, :])
    nc.finalize()
    return nc


def _wrap16(a):
    out = np.zeros((128, a.shape[1] // 16), a.dtype)
    for k in range(8):
        for p in range(16):
            out[16 * k + p] = a[k, p::16]
    return out


def _prep(tm1_input, v_init, weights, bias, tau_params, scale_excitatory,
          scale_inhibitory, source_indices, target_indices, type_ids,
          tm1_indices, steps):
    one = np.float32(1.0)
    weights = np.asarray(weights, np.float32)
    es = np.where(weights > 0, np.float32(scale_excitatory),
                  np.where(weights < 0, np.float32(scale_inhibitory), one))
    sw = (weights * es).astype(np.float32)

    type_ids = np.asarray(type_ids)
    tau = np.asarray(tau_params, np.float32)[type_ids]
    taup = np.concatenate([tau, np.full(N - NREAL, 1.0, np.float32)])
    is_tm1 = np.zeros(N, bool)
    tm1_indices = np.asarray(tm1_indices)
    is_tm1[tm1_indices] = True
    biasp = np.zeros(N, np.float32)
    biasp[:NREAL] = np.asarray(bias, np.float32)

    vc = np.zeros((B, N), np.float32)
    vc[:, :NREAL] = np.asarray(v_init, np.float32)
    vc[:, tm1_indices] = np.asarray(tm1_input, np.float32)

    order = np.argsort(target_indices, kind="stable")
    tsrc = np.asarray(source_indices)[order].astype(np.int64)
    tw = sw[order]
    ttgt = np.asarray(target_indices)[order].astype(np.int64)
    t_starts = np.searchsorted(ttgt, np.arange(N + 1, dtype=np.int64), side="left")

    F = np.zeros((128, 128), np.float32)
    for p in range(128):
        for m in range(128):
            if p // 16 == m // 16 and p % 8 == m % 8:
                F[p, m] = 1.0

    in_maps = []
    meta = []
    HC = ECOLS // 2
    for d in range(NDEV):
        idx1 = np.zeros((8, LCORE), np.int16)
        wmc = np.zeros((16, LCORE), np.float16)   # row g = 2k + h
        bpos = np.zeros((8, ECOLS), np.int16)
        col_of_t = np.zeros((8, MCORE), np.int64)
        for k in range(NCORES):
            t0 = d * NDEVT + k * MCORE
            e0, e1 = t_starts[t0], t_starts[t0 + MCORE]
            srcs = tsrc[e0:e1]
            ws = tw[e0:e1]
            counts = t_starts[t0 + 1:t0 + MCORE + 1] - t_starts[t0:t0 + MCORE]
            pos = np.cumsum(counts)              # extract position per target
            nslots = 1 + len(srcs)               # sentinel at slot 0
            assert nslots <= LCORE, f"core slots {nslots} > {LCORE}"
            idx1[k, 1:nslots] = (srcs % NH).astype(np.int16)
            half = np.zeros(LCORE, np.int64)
            wrow = np.zeros(LCORE, np.float32)
            half[1:nslots] = srcs // NH
            wrow[1:nslots] = ws
            for h in range(2):
                wmc[2 * k + h] = np.where(half == h, wrow, 0.0).astype(np.float16)
            # boundary extraction, chunked (real targets only; virtual
            # padding targets share the final pad column: syn there is
            # garbage but mdt=0 and v0=0 keep their state at 0)
            ids_k = d * NDEVT + k * MCORE + np.arange(MCORE)
            cchunk = pos // CHUNK
            clocal = pos % CHUNK
            ci = 0
            for c in range(NCHUNK):
                nhere = 0
                while ci < MCORE and cchunk[ci] == c:
                    if ids_k[ci] >= NREAL:
                        col_of_t[k, ci] = ECOLS - 1
                        ci += 1
                        continue
                    assert nhere < BCH - 1, f"chunk {c} boundary overflow"
                    bpos[k, c * BCH + nhere] = clocal[ci]
                    col_of_t[k, ci] = c * BCH + nhere
                    nhere += 1
                    ci += 1
                padv = bpos[k, c * BCH + nhere - 1] if nhere else 0
                bpos[k, c * BCH + nhere:(c + 1) * BCH] = padv
            assert ci == MCORE
        gids = (d * NDEVT + np.arange(NDEVT)).reshape(NCORES, MCORE)
        v0 = np.zeros((8 * B, ECOLS), np.float32)   # row 8k + b
        mdt = np.zeros((8, ECOLS), np.float32)      # row k
        bmt = np.zeros((8, ECOLS), np.float32)
        for k in range(NCORES):
            cols = col_of_t[k]
            ids = gids[k]
            upd = (~is_tm1[ids]) & (ids < NREAL)
            mvals = np.where(upd, DT / taup[ids], 0.0).astype(np.float32)
            mdt[k, cols] = mvals
            bmt[k, cols] = (mvals * biasp[ids]).astype(np.float32)
            for b in range(B):
                v0[8 * k + b, cols] = vc[b, ids]
        m = {
            "bidx": _wrap16(bpos), "idx3": _wrap16(col_of_t.astype(np.int16)),
        }
        ix_w = _wrap16(idx1)
        for j in range(16):
            m[f"ix{j}"] = np.ascontiguousarray(ix_w[:, j * 100:(j + 1) * 100])
        WQ = LCORE // 32
        for j in range(32):
            m[f"wm{j}"] = np.ascontiguousarray(wmc[:, j * WQ:(j + 1) * WQ])
        for b in range(B):
            vb = v0[np.arange(NCORES) * 8 + b]
            m[f"v0{2 * b}"] = np.ascontiguousarray(vb[:, 0:HC])
            m[f"v0{2 * b + 1}"] = np.ascontiguousarray(vb[:, HC:ECOLS])
        for j in range(4):
            m[f"fold{j}"] = np.ascontiguousarray(F[32 * j:32 * (j + 1), :])
        m["mdt0"] = np.ascontiguousarray(mdt[:, 0:HC])
        m["mdt1"] = np.ascontiguousarray(mdt[:, HC:ECOLS])
        m["bm0"] = np.ascontiguousarray(bmt[:, 0:HC])
        m["bm1"] = np.ascontiguousarray(bmt[:, HC:ECOLS])
        in_maps.append(m)
        meta.append(col_of_t)
    return in_maps, meta


def kernel(**inputs):
    steps = int(inputs["steps"])
    bias = np.asarray(inputs["bias"])
    with_bias = bool(np.any(bias != 0))
    in_maps, _meta = _prep(**inputs)
    if not with_bias:
        for m in in_maps:
            m.pop("bm0")
            m.pop("bm1")
    key = (steps, with_bias)
    if key not in _cache:
        _cache[key] = _build(steps, with_bias)
    nc = _cache[key]
    res = run_bass_kernel_spmd(nc, in_maps, list(range(NDEV)))
    out = np.zeros((B, NREAL), np.float32)
    for d in range(NDEV):
        sl = res.results[d]["vout"].astype(np.float32)
        lo = d * NDEVT
        hi = min(lo + NDEVT, NREAL)
        out[:, lo:hi] = sl[:, :hi - lo]
    return out


# revision 45
# speedup vs baseline: 1.1014x; 1.1014x over previous
"""Drosophila optic lobe circuit simulation on 8 Trainium2 NeuronCores.

Edge/target-sharded across 8 devices; batch rides partitions.
- N padded 49000->49152 = 8 dev x 8 cores x 768 targets.
- Gather tables: partition p = 16k + 8h + b holds r=relu(v) of batch b,
  source-half h ([24576] fp32). ap_gather per 1600-slot chunk fetches
  r[src] for all 8 batches; wrong-half rows masked by wmask=0.
- currents = gathered * wmask (fp16 static weights streamed from DRAM).
- Scatter-add = carried inclusive cumsum (tensor_tensor_scan) + boundary
  extraction (small ap_gather per chunk) + first difference.
- Source halves folded by a static 0/1 matmul; v updated in extract-column
  layout; r re-sharded to id order (ap_gather), AllGathered across devices,
  tables rebuilt by broadcast DMAs.

Host->device transfer over the axon tunnel is the wall-clock bottleneck
and scales with the LARGEST single parameter, not total bytes (arrays
stream in parallel). So inputs are shipped deduplicated (weights are
batch-invariant: 16 unique rows, not 128; v0 is half-invariant; mdt is
per-core) and split into ~200KB chunks. The r-table (12.6MB, formerly an
input) is built on device by running the publish path once before the
loop. Weights are expanded once into device DRAM (wmx) and streamed
per-chunk from there each step, keeping the steady-state instruction
stream unchanged.
"""

import numpy as np
import sys

sys.path.insert(0, "/opt/trn_rl_repo")

import jax

import concourse.bacc as bacc
import concourse.mybir as mybir
from concourse.tile import TileContext
from concourse.bass_utils import run_bass_kernel_spmd

# Cache XLA executables on disk so repeat dispatches skip the re-compile
# path (the jit closure inside run_bass_via_pjrt is fresh per call, so
# jax's in-memory caches never hit).
jax.config.update("jax_compilation_cache_dir", "/tmp/jax_cache_bass")
jax.config.update("jax_persistent_cache_min_compile_time_secs", 0.0)
jax.config.update("jax_persistent_cache_min_entry_size_bytes", 0)

NREAL = 49000
B = 8
DT = 0.1
NDEV = 8
N = 49152
NH = N // 2
NDEVT = N // NDEV          # 6144
NCORES = 8
MCORE = NDEVT // NCORES    # 768
NCHUNK = 8
CHUNK = 3200
LCORE = NCHUNK * CHUNK     # 25600
BCH = 128
ECOLS = NCHUNK * BCH       # 1024

_cache = {}


def _build(steps, with_bias, do_collective=True, do_publish=True, do_chunks=True,
           ag_shared=True, use_for_i=False, gprobe="none", agbufs=None):
    # use_for_i: tc.For_i hardware loops execute, but collective_compute
    # inside the loop body fails at runtime on this stack — keep unrolled.
    # gprobe: timing-only ap_gather scaling probes ("elems"/"idxs"), wrong numerics.
    nc = bacc.Bacc(None)
    f32, f16, i16 = mybir.dt.float32, mybir.dt.float16, mybir.dt.int16

    # compact, split inputs (transfer wall ~ largest single param)
    wm_in = [nc.declare_dram_parameter(f"wm{j}", [16, LCORE // 32], f16,
                                       isOutput=False) for j in range(32)]
    ix_in = [nc.declare_dram_parameter(f"ix{c}", [128, LCORE // 256], i16,
                                       isOutput=False) for c in range(16)]
    v0_in = [nc.declare_dram_parameter(f"v0{j}", [8, ECOLS // 2], f32,
                                       isOutput=False) for j in range(16)]
    fold_in = [nc.declare_dram_parameter(f"fold{j}", [32, 128], f32,
                                         isOutput=False) for j in range(4)]
    mdt_in = [nc.declare_dram_parameter(f"mdt{j}", [8, ECOLS // 2], f32,
                                        isOutput=False) for j in range(2)]
    if with_bias:
        bm_in = [nc.declare_dram_parameter(f"bm{j}", [8, ECOLS // 2], f32,
                                           isOutput=False) for j in range(2)]
    bidx_in = nc.declare_dram_parameter("bidx", [128, ECOLS // 16], i16, isOutput=False)
    idx3_in = nc.declare_dram_parameter("idx3", [128, MCORE // 16], i16, isOutput=False)
    vout = nc.declare_dram_parameter("vout", [B, NDEVT], f16, isOutput=True)

    with TileContext(nc) as tc:
        with (
            tc.tile_pool(name="big", bufs=1) as big,
            tc.tile_pool(name="gbuf", bufs=2) as gbuf,
            tc.tile_pool(name="wbuf", bufs=2) as wbuf,
            tc.tile_pool(name="cs", bufs=2) as csp,
            tc.tile_pool(name="small", bufs=1) as small,
            tc.tile_pool(name="psum", bufs=1, space="PSUM") as psum,
            tc.tile_pool(name="dram", bufs=1, space="DRAM") as dram,
            tc.tile_pool(name="agpool",
                         bufs=agbufs or (2 if use_for_i else max(steps, 1)),
                         space="DRAM") as agp,
        ):
            tbl = big.tile([128, NH], f32, tag="tbl")
            idx1 = small.tile([128, LCORE // 16], i16, tag="idx1")
            bidx = small.tile([128, ECOLS // 16], i16, tag="bidx")
            idx3 = small.tile([128, MCORE // 16], i16, tag="idx3")
            v = small.tile([128, ECOLS], f32, tag="v")
            mdt = small.tile([128, ECOLS], f32, tag="mdt")
            fold = small.tile([128, 128], f32, tag="fold")
            bm = small.tile([128, ECOLS], f32, tag="bm") if with_bias else None
            E = small.tile([128, ECOLS], f32, tag="E")
            syn = small.tile([128, ECOLS], f32, tag="syn")
            rslab = small.tile([128, MCORE], f32, tag="rslab")

            slab_d = dram.tile([B, NDEVT], f32)
            wmx = dram.tile([128, LCORE], f16)

            # ---- input loads / on-device expansion (one-time) ----
            for j in range(16):
                nc.sync.dma_start(out=idx1[:, j * 100:(j + 1) * 100], in_=ix_in[j][:])
            nc.sync.dma_start(out=bidx[:], in_=bidx_in[:])
            nc.sync.dma_start(out=idx3[:], in_=idx3_in[:])
            for j in range(4):
                nc.sync.dma_start(out=fold[32 * j:32 * (j + 1), :], in_=fold_in[j][:])
            HC = ECOLS // 2
            mdtv = mdt[:].rearrange("(k r) n -> k r n", k=8)
            for r in range(16):
                nc.sync.dma_start(out=mdtv[:, r, 0:HC], in_=mdt_in[0][:])
                nc.sync.dma_start(out=mdtv[:, r, HC:ECOLS], in_=mdt_in[1][:])
            if with_bias:
                bmv = bm[:].rearrange("(k r) n -> k r n", k=8)
                for r in range(16):
                    nc.sync.dma_start(out=bmv[:, r, 0:HC], in_=bm_in[0][:])
                    nc.sync.dma_start(out=bmv[:, r, HC:ECOLS], in_=bm_in[1][:])
            vv = v[:].rearrange("(k h b) n -> k h b n", k=8, h=2)
            for h in range(2):
                for b in range(B):
                    nc.sync.dma_start(out=vv[:, h, b, 0:HC], in_=v0_in[2 * b][:])
                    nc.sync.dma_start(out=vv[:, h, b, HC:ECOLS], in_=v0_in[2 * b + 1][:])
            # expand weights (batch-invariant rows) into DRAM wmx once,
            # in param-width units (chunk-size agnostic)
            WQ = LCORE // 32          # 800 cols per wm param
            for t in range(16):
                w = wbuf.tile([128, 2 * WQ], f16, tag="wst")
                wv = w[:].rearrange("(g b) n -> g b n", g=16)
                for b in range(B):
                    nc.sync.dma_start(out=wv[:, b, 0:WQ], in_=wm_in[2 * t][:])
                    nc.sync.dma_start(out=wv[:, b, WQ:2 * WQ], in_=wm_in[2 * t + 1][:])
                nc.sync.dma_start(out=wmx[:, t * 2 * WQ:(t + 1) * 2 * WQ], in_=w[:])

            def publish():
                # r = relu(v) -> id-order slab -> DRAM -> AllGather(f32) ->
                # 16 broadcast DMAs rebuild the replicated table. Fewer
                # instructions beats the PE-matmul rebuild: per-instruction
                # dispatch overhead (~25us) dominates at this size.
                nc.scalar.activation(syn[:], v[:], mybir.ActivationFunctionType.Relu)
                nc.gpsimd.ap_gather(out_ap=rslab[:], in_ap=syn[:], idxs_ap=idx3[:],
                                    channels=128, num_elems=ECOLS, d=1, num_idxs=MCORE)
                for k in range(NCORES):
                    nc.sync.dma_start(out=slab_d[:, k * MCORE:(k + 1) * MCORE],
                                      in_=rslab[16 * k:16 * k + 8, :])
                if ag_shared:
                    ag_d = agp.tile([NDEV * B, NDEVT], f32, addr_space="Shared", tag="ag")
                else:
                    ag_d = agp.tile([NDEV * B, NDEVT], f32, tag="ag")
                if do_collective:
                    nc.gpsimd.collective_compute(
                        "AllGather", mybir.AluOpType.bypass,
                        replica_groups=[list(range(NDEV))],
                        ins=[slab_d[:]], outs=[ag_d[:]],
                    )
                agv = ag_d[:].rearrange("(d b) n -> d b n", d=NDEV)
                for h in range(2):
                    for b in range(B):
                        nc.sync.dma_start(
                            out=tbl[:].rearrange("(k r) n -> k r n", k=8)[:, 8 * h + b, :],
                            in_=agv[4 * h:4 * h + 4, b, :][None]
                                .to_broadcast([8, 4, NDEVT]),
                        )

            def step_body(last):
                if not do_chunks:
                    if not last and do_publish:
                        publish()
                    return
                s_tiles = {}
                for c in range(NCHUNK):
                    g = gbuf.tile([128, CHUNK], f32, tag="g")
                    w = wbuf.tile([128, CHUNK], f16, tag="w")
                    s = csp.tile([128, CHUNK], f32, tag="s")
                    s_tiles[c] = s
                    nc.sync.dma_start(out=w[:], in_=wmx[:, c * CHUNK:(c + 1) * CHUNK])
                    if gprobe == "elems":
                        # timing probe: tiny table, same idx count
                        nc.gpsimd.ap_gather(
                            out_ap=g[:], in_ap=tbl[:, 0:1600],
                            idxs_ap=idx1[:, c * (CHUNK // 16):(c + 1) * (CHUNK // 16)],
                            channels=128, num_elems=1600, d=1, num_idxs=CHUNK,
                        )
                    elif gprobe == "idxs":
                        # timing probe: quarter idx count, full table
                        nc.gpsimd.ap_gather(
                            out_ap=g[:, 0:400], in_ap=tbl[:],
                            idxs_ap=idx1[:, c * (CHUNK // 16):c * (CHUNK // 16) + 25],
                            channels=128, num_elems=NH, d=1, num_idxs=400,
                        )
                    else:
                        nc.gpsimd.ap_gather(
                            out_ap=g[:], in_ap=tbl[:],
                            idxs_ap=idx1[:, c * (CHUNK // 16):(c + 1) * (CHUNK // 16)],
                            channels=128, num_elems=NH, d=1, num_idxs=CHUNK,
                        )
                    # extract previous chunk's boundaries (after this gather so
                    # POOL doesn't stall on the DVE scan)
                    sp = s_tiles.pop(c - 1) if c >= 1 else None
                    if c >= 1:
                        nc.gpsimd.ap_gather(
                            out_ap=E[:, (c - 1) * BCH:c * BCH], in_ap=sp[:],
                            idxs_ap=bidx[:, (c - 1) * (BCH // 16):c * (BCH // 16)],
                            channels=128, num_elems=CHUNK, d=1, num_idxs=BCH,
                        )
                    nc.vector.tensor_tensor(out=g[:], in0=g[:], in1=w[:],
                                            op=mybir.AluOpType.mult)
                    init = 0.0 if c == 0 else sp[:, CHUNK - 1:CHUNK]
                    nc.vector.tensor_tensor_scan(
                        out=s[:], data0=g[:], data1=g[:], initial=init,
                        op0=mybir.AluOpType.add, op1=mybir.AluOpType.bypass,
                    )
                c = NCHUNK
                sp = s_tiles.pop(c - 1)
                nc.gpsimd.ap_gather(
                    out_ap=E[:, (c - 1) * BCH:c * BCH], in_ap=sp[:],
                    idxs_ap=bidx[:, (c - 1) * (BCH // 16):c * (BCH // 16)],
                    channels=128, num_elems=CHUNK, d=1, num_idxs=BCH,
                )
                # fold halves: Ef = fold.T @ E
                ef = psum.tile([128, ECOLS], f32, tag="ef")
                for j in range(0, ECOLS, 512):
                    jw = min(512, ECOLS - j)
                    nc.tensor.matmul(out=ef[:, j:j + jw], lhsT=fold[:],
                                     rhs=E[:, j:j + jw], start=True, stop=True)
                # segment sums by first difference (via SBUF copy of ef)
                nc.vector.tensor_copy(out=syn[:], in_=ef[:])
                nc.vector.tensor_copy(out=E[:, 0:1], in_=syn[:, 0:1])
                nc.vector.tensor_tensor(out=E[:, 1:ECOLS], in0=syn[:, 1:ECOLS],
                                        in1=syn[:, 0:ECOLS - 1],
                                        op=mybir.AluOpType.subtract)
                # v += mdt * (E - v) (+ bm)
                nc.vector.tensor_tensor(out=E[:], in0=E[:], in1=v[:],
                                        op=mybir.AluOpType.subtract)
                nc.vector.tensor_tensor(out=E[:], in0=E[:], in1=mdt[:],
                                        op=mybir.AluOpType.mult)
                nc.vector.tensor_tensor(out=v[:], in0=v[:], in1=E[:],
                                        op=mybir.AluOpType.add)
                if with_bias:
                    nc.vector.tensor_tensor(out=v[:], in0=v[:], in1=bm[:],
                                            op=mybir.AluOpType.add)
                if not last and do_publish:
                    publish()

            if steps >= 1:
                publish()
                if use_for_i:
                    if steps > 1:
                        with tc.For_i(0, steps - 1):
                            step_body(last=False)
                    step_body(last=True)
                else:
                    for s in range(steps):
                        step_body(last=(s == steps - 1))

            nc.gpsimd.ap_gather(out_ap=rslab[:], in_ap=v[:], idxs_ap=idx3[:],
                                channels=128, num_elems=ECOLS, d=1, num_idxs=MCORE)
            vslab16 = small.tile([128, MCORE], f16, tag="vslab16")
            nc.vector.tensor_copy(out=vslab16[:], in_=rslab[:])
            for k in range(NCORES):
                nc.sync.dma_start(out=vout[:, k * MCORE:(k + 1) * MCORE],
                                  in_=vslab16[16 * k:16 * k + 8, :])
    nc.finalize()
    return nc


def _wrap16(a):
    out = np.zeros((128, a.shape[1] // 16), a.dtype)
    for k in range(8):
        for p in range(16):
            out[16 * k + p] = a[k, p::16]
    return out


def _prep(tm1_input, v_init, weights, bias, tau_params, scale_excitatory,
          scale_inhibitory, source_indices, target_indices, type_ids,
          tm1_indices, steps):
    one = np.float32(1.0)
    weights = np.asarray(weights, np.float32)
    es = np.where(weights > 0, np.float32(scale_excitatory),
                  np.where(weights < 0, np.float32(scale_inhibitory), one))
    sw = (weights * es).astype(np.float32)

    type_ids = np.asarray(type_ids)
    tau = np.asarray(tau_params, np.float32)[type_ids]
    taup = np.concatenate([tau, np.full(N - NREAL, 1.0, np.float32)])
    is_tm1 = np.zeros(N, bool)
    tm1_indices = np.asarray(tm1_indices)
    is_tm1[tm1_indices] = True
    biasp = np.zeros(N, np.float32)
    biasp[:NREAL] = np.asarray(bias, np.float32)

    vc = np.zeros((B, N), np.float32)
    vc[:, :NREAL] = np.asarray(v_init, np.float32)
    vc[:, tm1_indices] = np.asarray(tm1_input, np.float32)

    order = np.argsort(target_indices, kind="stable")
    tsrc = np.asarray(source_indices)[order].astype(np.int64)
    tw = sw[order]
    ttgt = np.asarray(target_indices)[order].astype(np.int64)
    t_starts = np.searchsorted(ttgt, np.arange(N + 1, dtype=np.int64), side="left")

    F = np.zeros((128, 128), np.float32)
    for p in range(128):
        for m in range(128):
            if p // 16 == m // 16 and p % 8 == m % 8:
                F[p, m] = 1.0

    in_maps = []
    meta = []
    HC = ECOLS // 2
    for d in range(NDEV):
        idx1 = np.zeros((8, LCORE), np.int16)
        wmc = np.zeros((16, LCORE), np.float16)   # row g = 2k + h
        bpos = np.zeros((8, ECOLS), np.int16)
        col_of_t = np.zeros((8, MCORE), np.int64)
        for k in range(NCORES):
            t0 = d * NDEVT + k * MCORE
            e0, e1 = t_starts[t0], t_starts[t0 + MCORE]
            srcs = tsrc[e0:e1]
            ws = tw[e0:e1]
            counts = t_starts[t0 + 1:t0 + MCORE + 1] - t_starts[t0:t0 + MCORE]
            pos = np.cumsum(counts)              # extract position per target
            nslots = 1 + len(srcs)               # sentinel at slot 0
            assert nslots <= LCORE, f"core slots {nslots} > {LCORE}"
            idx1[k, 1:nslots] = (srcs % NH).astype(np.int16)
            half = np.zeros(LCORE, np.int64)
            wrow = np.zeros(LCORE, np.float32)
            half[1:nslots] = srcs // NH
            wrow[1:nslots] = ws
            for h in range(2):
                wmc[2 * k + h] = np.where(half == h, wrow, 0.0).astype(np.float16)
            # boundary extraction, chunked (real targets only; virtual
            # padding targets share the final pad column: syn there is
            # garbage but mdt=0 and v0=0 keep their state at 0)
            ids_k = d * NDEVT + k * MCORE + np.arange(MCORE)
            cchunk = pos // CHUNK
            clocal = pos % CHUNK
            ci = 0
            for c in range(NCHUNK):
                nhere = 0
                while ci < MCORE and cchunk[ci] == c:
                    if ids_k[ci] >= NREAL:
                        col_of_t[k, ci] = ECOLS - 1
                        ci += 1
                        continue
                    assert nhere < BCH - 1, f"chunk {c} boundary overflow"
                    bpos[k, c * BCH + nhere] = clocal[ci]
                    col_of_t[k, ci] = c * BCH + nhere
                    nhere += 1
                    ci += 1
                padv = bpos[k, c * BCH + nhere - 1] if nhere else 0
                bpos[k, c * BCH + nhere:(c + 1) * BCH] = padv
            assert ci == MCORE
        gids = (d * NDEVT + np.arange(NDEVT)).reshape(NCORES, MCORE)
        v0 = np.zeros((8 * B, ECOLS), np.float32)   # row 8k + b
        mdt = np.zeros((8, ECOLS), np.float32)      # row k
        bmt = np.zeros((8, ECOLS), np.float32)
        for k in range(NCORES):
            cols = col_of_t[k]
            ids = gids[k]
            upd = (~is_tm1[ids]) & (ids < NREAL)
            mvals = np.where(upd, DT / taup[ids], 0.0).astype(np.float32)
            mdt[k, cols] = mvals
            bmt[k, cols] = (mvals * biasp[ids]).astype(np.float32)
            for b in range(B):
                v0[8 * k + b, cols] = vc[b, ids]
        m = {
            "bidx": _wrap16(bpos), "idx3": _wrap16(col_of_t.astype(np.int16)),
        }
        ix_w = _wrap16(idx1)
        for j in range(16):
            m[f"ix{j}"] = np.ascontiguousarray(ix_w[:, j * 100:(j + 1) * 100])
        WQ = LCORE // 32
        for j in range(32):
            m[f"wm{j}"] = np.ascontiguousarray(wmc[:, j * WQ:(j + 1) * WQ])
        for b in range(B):
            vb = v0[np.arange(NCORES) * 8 + b]
            m[f"v0{2 * b}"] = np.ascontiguousarray(vb[:, 0:HC])
            m[f"v0{2 * b + 1}"] = np.ascontiguousarray(vb[:, HC:ECOLS])
        for j in range(4):
            m[f"fold{j}"] = np.ascontiguousarray(F[32 * j:32 * (j + 1), :])
        m["mdt0"] = np.ascontiguousarray(mdt[:, 0:HC])
        m["mdt1"] = np.ascontiguousarray(mdt[:, HC:ECOLS])
        m["bm0"] = np.ascontiguousarray(bmt[:, 0:HC])
        m["bm1"] = np.ascontiguousarray(bmt[:, HC:ECOLS])
        in_maps.append(m)
        meta.append(col_of_t)
    return in_maps, meta


def kernel(**inputs):
    steps = int(inputs["steps"])
    bias = np.asarray(inputs["bias"])
    with_bias = bool(np.any(bias != 0))
    in_maps, _meta = _prep(**inputs)
    if not with_bias:
        for m in in_maps:
            m.pop("bm0")
            m.pop("bm1")
    key = (steps, with_bias)
    if key not in _cache:
        _cache[key] = _build(steps, with_bias)
    nc = _cache[key]
    res = run_bass_kernel_spmd(nc, in_maps, list(range(NDEV)))
    out = np.zeros((B, NREAL), np.float32)
    for d in range(NDEV):
        sl = res.results[d]["vout"].astype(np.float32)
        lo = d * NDEVT
        hi = min(lo + NDEVT, NREAL)
        out[:, lo:hi] = sl[:, :hi - lo]
    return out


# revision 46
# speedup vs baseline: 1.1424x; 1.0372x over previous
"""Drosophila optic lobe circuit simulation on 8 Trainium2 NeuronCores.

Edge/target-sharded across 8 devices; batch rides partitions.
- N padded 49000->49152 = 8 dev x 8 cores x 768 targets.
- Gather tables: partition p = 16k + 8h + b holds r=relu(v) of batch b,
  source-half h ([24576] fp32). ap_gather per 1600-slot chunk fetches
  r[src] for all 8 batches; wrong-half rows masked by wmask=0.
- currents = gathered * wmask (fp16 static weights streamed from DRAM).
- Scatter-add = carried inclusive cumsum (tensor_tensor_scan) + boundary
  extraction (small ap_gather per chunk) + first difference.
- Source halves folded by a static 0/1 matmul; v updated in extract-column
  layout; r re-sharded to id order (ap_gather), AllGathered across devices,
  tables rebuilt by broadcast DMAs.

Host->device transfer over the axon tunnel is the wall-clock bottleneck
and scales with the LARGEST single parameter, not total bytes (arrays
stream in parallel). So inputs are shipped deduplicated (weights are
batch-invariant: 16 unique rows, not 128; v0 is half-invariant; mdt is
per-core) and split into ~200KB chunks. The r-table (12.6MB, formerly an
input) is built on device by running the publish path once before the
loop. Weights are expanded once into device DRAM (wmx) and streamed
per-chunk from there each step, keeping the steady-state instruction
stream unchanged.
"""

import numpy as np
import sys

sys.path.insert(0, "/opt/trn_rl_repo")

import jax

import concourse.bacc as bacc
import concourse.mybir as mybir
from concourse.tile import TileContext
from concourse.bass_utils import run_bass_kernel_spmd

# Cache XLA executables on disk so repeat dispatches skip the re-compile
# path (the jit closure inside run_bass_via_pjrt is fresh per call, so
# jax's in-memory caches never hit).
jax.config.update("jax_compilation_cache_dir", "/tmp/jax_cache_bass")
jax.config.update("jax_persistent_cache_min_compile_time_secs", 0.0)
jax.config.update("jax_persistent_cache_min_entry_size_bytes", 0)

NREAL = 49000
B = 8
DT = 0.1
NDEV = 8
N = 49152
NH = N // 2
NDEVT = N // NDEV          # 6144
NCORES = 8
MCORE = NDEVT // NCORES    # 768
NCHUNK = 8
CHUNK = 3200
LCORE = NCHUNK * CHUNK     # 25600
BCH = 128
ECOLS = NCHUNK * BCH       # 1024

_cache = {}


def _build(steps, with_bias, do_collective=True, do_publish=True, do_chunks=True,
           ag_shared=True, use_for_i=False, gprobe="none", agbufs=None):
    # use_for_i: tc.For_i hardware loops execute, but collective_compute
    # inside the loop body fails at runtime on this stack — keep unrolled.
    # gprobe: timing-only ap_gather scaling probes ("elems"/"idxs"), wrong numerics.
    nc = bacc.Bacc(None)
    f32, f16, i16 = mybir.dt.float32, mybir.dt.float16, mybir.dt.int16

    # compact, split inputs (transfer wall ~ largest single param)
    wm_in = [nc.declare_dram_parameter(f"wm{j}", [16, LCORE // 32], f16,
                                       isOutput=False) for j in range(32)]
    ix_in = [nc.declare_dram_parameter(f"ix{c}", [128, LCORE // 256], i16,
                                       isOutput=False) for c in range(16)]
    v0_in = [nc.declare_dram_parameter(f"v0{j}", [8, ECOLS // 2], f32,
                                       isOutput=False) for j in range(16)]
    fold_in = [nc.declare_dram_parameter(f"fold{j}", [32, 128], f32,
                                         isOutput=False) for j in range(4)]
    mdt_in = [nc.declare_dram_parameter(f"mdt{j}", [8, ECOLS // 2], f32,
                                        isOutput=False) for j in range(2)]
    if with_bias:
        bm_in = [nc.declare_dram_parameter(f"bm{j}", [8, ECOLS // 2], f32,
                                           isOutput=False) for j in range(2)]
    bidx_in = nc.declare_dram_parameter("bidx", [128, ECOLS // 16], i16, isOutput=False)
    idx3_in = nc.declare_dram_parameter("idx3", [128, MCORE // 16], i16, isOutput=False)
    vout = nc.declare_dram_parameter("vout", [B, NDEVT], f16, isOutput=True)

    with TileContext(nc) as tc:
        with (
            tc.tile_pool(name="big", bufs=1) as big,
            tc.tile_pool(name="gbuf", bufs=2) as gbuf,
            tc.tile_pool(name="wbuf", bufs=2) as wbuf,
            tc.tile_pool(name="cs", bufs=2) as csp,
            tc.tile_pool(name="small", bufs=1) as small,
            tc.tile_pool(name="psum", bufs=1, space="PSUM") as psum,
            tc.tile_pool(name="dram", bufs=1, space="DRAM") as dram,
            tc.tile_pool(name="agpool",
                         bufs=agbufs or (2 if use_for_i else max(steps, 1)),
                         space="DRAM") as agp,
        ):
            tbl = big.tile([128, NH], f32, tag="tbl")
            idx1 = small.tile([128, LCORE // 16], i16, tag="idx1")
            bidx = small.tile([128, ECOLS // 16], i16, tag="bidx")
            idx3 = small.tile([128, MCORE // 16], i16, tag="idx3")
            v = small.tile([128, ECOLS], f32, tag="v")
            mdt = small.tile([128, ECOLS], f32, tag="mdt")
            fold = small.tile([128, 128], f32, tag="fold")
            bm = small.tile([128, ECOLS], f32, tag="bm") if with_bias else None
            E = small.tile([128, ECOLS], f32, tag="E")
            syn = small.tile([128, ECOLS], f32, tag="syn")
            rslab = small.tile([128, MCORE], f32, tag="rslab")

            slab_d = dram.tile([B, NDEVT], f32)
            wmx = dram.tile([128, LCORE], f16)

            # ---- input loads / on-device expansion (one-time) ----
            for j in range(16):
                nc.sync.dma_start(out=idx1[:, j * 100:(j + 1) * 100], in_=ix_in[j][:])
            nc.sync.dma_start(out=bidx[:], in_=bidx_in[:])
            nc.sync.dma_start(out=idx3[:], in_=idx3_in[:])
            for j in range(4):
                nc.sync.dma_start(out=fold[32 * j:32 * (j + 1), :], in_=fold_in[j][:])
            HC = ECOLS // 2
            mdtv = mdt[:].rearrange("(k r) n -> k r n", k=8)
            for r in range(16):
                nc.sync.dma_start(out=mdtv[:, r, 0:HC], in_=mdt_in[0][:])
                nc.sync.dma_start(out=mdtv[:, r, HC:ECOLS], in_=mdt_in[1][:])
            if with_bias:
                bmv = bm[:].rearrange("(k r) n -> k r n", k=8)
                for r in range(16):
                    nc.sync.dma_start(out=bmv[:, r, 0:HC], in_=bm_in[0][:])
                    nc.sync.dma_start(out=bmv[:, r, HC:ECOLS], in_=bm_in[1][:])
            vv = v[:].rearrange("(k h b) n -> k h b n", k=8, h=2)
            for h in range(2):
                for b in range(B):
                    nc.sync.dma_start(out=vv[:, h, b, 0:HC], in_=v0_in[2 * b][:])
                    nc.sync.dma_start(out=vv[:, h, b, HC:ECOLS], in_=v0_in[2 * b + 1][:])
            # expand weights (batch-invariant rows) into DRAM wmx once,
            # in param-width units (chunk-size agnostic)
            WQ = LCORE // 32          # 800 cols per wm param
            for t in range(16):
                w = wbuf.tile([128, 2 * WQ], f16, tag="wst")
                wv = w[:].rearrange("(g b) n -> g b n", g=16)
                for b in range(B):
                    nc.sync.dma_start(out=wv[:, b, 0:WQ], in_=wm_in[2 * t][:])
                    nc.sync.dma_start(out=wv[:, b, WQ:2 * WQ], in_=wm_in[2 * t + 1][:])
                nc.sync.dma_start(out=wmx[:, t * 2 * WQ:(t + 1) * 2 * WQ], in_=w[:])

            def publish():
                # r = relu(v) -> id-order slab -> DRAM -> AllGather(f32) ->
                # 16 broadcast DMAs rebuild the replicated table. Fewer
                # instructions beats the PE-matmul rebuild: per-instruction
                # dispatch overhead (~25us) dominates at this size.
                nc.scalar.activation(syn[:], v[:], mybir.ActivationFunctionType.Relu)
                nc.gpsimd.ap_gather(out_ap=rslab[:], in_ap=syn[:], idxs_ap=idx3[:],
                                    channels=128, num_elems=ECOLS, d=1, num_idxs=MCORE)
                for k in range(NCORES):
                    eng = nc.sync if k % 2 == 0 else nc.scalar
                    eng.dma_start(out=slab_d[:, k * MCORE:(k + 1) * MCORE],
                                  in_=rslab[16 * k:16 * k + 8, :])
                if ag_shared:
                    ag_d = agp.tile([NDEV * B, NDEVT], f32, addr_space="Shared", tag="ag")
                else:
                    ag_d = agp.tile([NDEV * B, NDEVT], f32, tag="ag")
                if do_collective:
                    nc.gpsimd.collective_compute(
                        "AllGather", mybir.AluOpType.bypass,
                        replica_groups=[list(range(NDEV))],
                        ins=[slab_d[:]], outs=[ag_d[:]],
                    )
                agv = ag_d[:].rearrange("(d b) n -> d b n", d=NDEV)
                for h in range(2):
                    for b in range(B):
                        eng = nc.sync if b % 2 == 0 else nc.scalar
                        eng.dma_start(
                            out=tbl[:].rearrange("(k r) n -> k r n", k=8)[:, 8 * h + b, :],
                            in_=agv[4 * h:4 * h + 4, b, :][None]
                                .to_broadcast([8, 4, NDEVT]),
                        )

            def step_body(last):
                if not do_chunks:
                    if not last and do_publish:
                        publish()
                    return
                s_tiles = {}
                w2 = None
                for c in range(NCHUNK):
                    g = gbuf.tile([128, CHUNK], f32, tag="g")
                    if c % 2 == 0:
                        w2 = wbuf.tile([128, 2 * CHUNK], f16, tag="w")
                        nc.sync.dma_start(
                            out=w2[:], in_=wmx[:, c * CHUNK:(c + 2) * CHUNK])
                    w = w2[:, (c % 2) * CHUNK:(c % 2 + 1) * CHUNK]
                    s = csp.tile([128, CHUNK], f32, tag="s")
                    s_tiles[c] = s
                    if gprobe == "elems":
                        # timing probe: tiny table, same idx count
                        nc.gpsimd.ap_gather(
                            out_ap=g[:], in_ap=tbl[:, 0:1600],
                            idxs_ap=idx1[:, c * (CHUNK // 16):(c + 1) * (CHUNK // 16)],
                            channels=128, num_elems=1600, d=1, num_idxs=CHUNK,
                        )
                    elif gprobe == "idxs":
                        # timing probe: quarter idx count, full table
                        nc.gpsimd.ap_gather(
                            out_ap=g[:, 0:400], in_ap=tbl[:],
                            idxs_ap=idx1[:, c * (CHUNK // 16):c * (CHUNK // 16) + 25],
                            channels=128, num_elems=NH, d=1, num_idxs=400,
                        )
                    else:
                        nc.gpsimd.ap_gather(
                            out_ap=g[:], in_ap=tbl[:],
                            idxs_ap=idx1[:, c * (CHUNK // 16):(c + 1) * (CHUNK // 16)],
                            channels=128, num_elems=NH, d=1, num_idxs=CHUNK,
                        )
                    # extract previous chunk's boundaries (after this gather so
                    # POOL doesn't stall on the DVE scan)
                    sp = s_tiles.pop(c - 1) if c >= 1 else None
                    if c >= 1:
                        nc.gpsimd.ap_gather(
                            out_ap=E[:, (c - 1) * BCH:c * BCH], in_ap=sp[:],
                            idxs_ap=bidx[:, (c - 1) * (BCH // 16):c * (BCH // 16)],
                            channels=128, num_elems=CHUNK, d=1, num_idxs=BCH,
                        )
                    nc.vector.tensor_tensor(out=g[:], in0=g[:], in1=w,
                                            op=mybir.AluOpType.mult)
                    init = 0.0 if c == 0 else sp[:, CHUNK - 1:CHUNK]
                    nc.vector.tensor_tensor_scan(
                        out=s[:], data0=g[:], data1=g[:], initial=init,
                        op0=mybir.AluOpType.add, op1=mybir.AluOpType.bypass,
                    )
                c = NCHUNK
                sp = s_tiles.pop(c - 1)
                nc.gpsimd.ap_gather(
                    out_ap=E[:, (c - 1) * BCH:c * BCH], in_ap=sp[:],
                    idxs_ap=bidx[:, (c - 1) * (BCH // 16):c * (BCH // 16)],
                    channels=128, num_elems=CHUNK, d=1, num_idxs=BCH,
                )
                # fold halves: Ef = fold.T @ E
                ef = psum.tile([128, ECOLS], f32, tag="ef")
                for j in range(0, ECOLS, 512):
                    jw = min(512, ECOLS - j)
                    nc.tensor.matmul(out=ef[:, j:j + jw], lhsT=fold[:],
                                     rhs=E[:, j:j + jw], start=True, stop=True)
                # segment sums by first difference (via SBUF copy of ef)
                nc.vector.tensor_copy(out=syn[:], in_=ef[:])
                nc.vector.tensor_copy(out=E[:, 0:1], in_=syn[:, 0:1])
                nc.vector.tensor_tensor(out=E[:, 1:ECOLS], in0=syn[:, 1:ECOLS],
                                        in1=syn[:, 0:ECOLS - 1],
                                        op=mybir.AluOpType.subtract)
                # v += mdt * (E - v) (+ bm)
                nc.vector.tensor_tensor(out=E[:], in0=E[:], in1=v[:],
                                        op=mybir.AluOpType.subtract)
                nc.vector.tensor_tensor(out=E[:], in0=E[:], in1=mdt[:],
                                        op=mybir.AluOpType.mult)
                nc.vector.tensor_tensor(out=v[:], in0=v[:], in1=E[:],
                                        op=mybir.AluOpType.add)
                if with_bias:
                    nc.vector.tensor_tensor(out=v[:], in0=v[:], in1=bm[:],
                                            op=mybir.AluOpType.add)
                if not last and do_publish:
                    publish()

            if steps >= 1:
                publish()
                if use_for_i:
                    if steps > 1:
                        with tc.For_i(0, steps - 1):
                            step_body(last=False)
                    step_body(last=True)
                else:
                    for s in range(steps):
                        step_body(last=(s == steps - 1))

            nc.gpsimd.ap_gather(out_ap=rslab[:], in_ap=v[:], idxs_ap=idx3[:],
                                channels=128, num_elems=ECOLS, d=1, num_idxs=MCORE)
            vslab16 = small.tile([128, MCORE], f16, tag="vslab16")
            nc.vector.tensor_copy(out=vslab16[:], in_=rslab[:])
            for k in range(NCORES):
                nc.sync.dma_start(out=vout[:, k * MCORE:(k + 1) * MCORE],
                                  in_=vslab16[16 * k:16 * k + 8, :])
    nc.finalize()
    return nc


def _wrap16(a):
    out = np.zeros((128, a.shape[1] // 16), a.dtype)
    for k in range(8):
        for p in range(16):
            out[16 * k + p] = a[k, p::16]
    return out


def _prep(tm1_input, v_init, weights, bias, tau_params, scale_excitatory,
          scale_inhibitory, source_indices, target_indices, type_ids,
          tm1_indices, steps):
    one = np.float32(1.0)
    weights = np.asarray(weights, np.float32)
    es = np.where(weights > 0, np.float32(scale_excitatory),
                  np.where(weights < 0, np.float32(scale_inhibitory), one))
    sw = (weights * es).astype(np.float32)

    type_ids = np.asarray(type_ids)
    tau = np.asarray(tau_params, np.float32)[type_ids]
    taup = np.concatenate([tau, np.full(N - NREAL, 1.0, np.float32)])
    is_tm1 = np.zeros(N, bool)
    tm1_indices = np.asarray(tm1_indices)
    is_tm1[tm1_indices] = True
    biasp = np.zeros(N, np.float32)
    biasp[:NREAL] = np.asarray(bias, np.float32)

    vc = np.zeros((B, N), np.float32)
    vc[:, :NREAL] = np.asarray(v_init, np.float32)
    vc[:, tm1_indices] = np.asarray(tm1_input, np.float32)

    order = np.argsort(target_indices, kind="stable")
    tsrc = np.asarray(source_indices)[order].astype(np.int64)
    tw = sw[order]
    ttgt = np.asarray(target_indices)[order].astype(np.int64)
    t_starts = np.searchsorted(ttgt, np.arange(N + 1, dtype=np.int64), side="left")

    F = np.zeros((128, 128), np.float32)
    for p in range(128):
        for m in range(128):
            if p // 16 == m // 16 and p % 8 == m % 8:
                F[p, m] = 1.0

    in_maps = []
    meta = []
    HC = ECOLS // 2
    for d in range(NDEV):
        idx1 = np.zeros((8, LCORE), np.int16)
        wmc = np.zeros((16, LCORE), np.float16)   # row g = 2k + h
        bpos = np.zeros((8, ECOLS), np.int16)
        col_of_t = np.zeros((8, MCORE), np.int64)
        for k in range(NCORES):
            t0 = d * NDEVT + k * MCORE
            e0, e1 = t_starts[t0], t_starts[t0 + MCORE]
            srcs = tsrc[e0:e1]
            ws = tw[e0:e1]
            counts = t_starts[t0 + 1:t0 + MCORE + 1] - t_starts[t0:t0 + MCORE]
            pos = np.cumsum(counts)              # extract position per target
            nslots = 1 + len(srcs)               # sentinel at slot 0
            assert nslots <= LCORE, f"core slots {nslots} > {LCORE}"
            idx1[k, 1:nslots] = (srcs % NH).astype(np.int16)
            half = np.zeros(LCORE, np.int64)
            wrow = np.zeros(LCORE, np.float32)
            half[1:nslots] = srcs // NH
            wrow[1:nslots] = ws
            for h in range(2):
                wmc[2 * k + h] = np.where(half == h, wrow, 0.0).astype(np.float16)
            # boundary extraction, chunked (real targets only; virtual
            # padding targets share the final pad column: syn there is
            # garbage but mdt=0 and v0=0 keep their state at 0)
            ids_k = d * NDEVT + k * MCORE + np.arange(MCORE)
            cchunk = pos // CHUNK
            clocal = pos % CHUNK
            ci = 0
            for c in range(NCHUNK):
                nhere = 0
                while ci < MCORE and cchunk[ci] == c:
                    if ids_k[ci] >= NREAL:
                        col_of_t[k, ci] = ECOLS - 1
                        ci += 1
                        continue
                    assert nhere < BCH - 1, f"chunk {c} boundary overflow"
                    bpos[k, c * BCH + nhere] = clocal[ci]
                    col_of_t[k, ci] = c * BCH + nhere
                    nhere += 1
                    ci += 1
                padv = bpos[k, c * BCH + nhere - 1] if nhere else 0
                bpos[k, c * BCH + nhere:(c + 1) * BCH] = padv
            assert ci == MCORE
        gids = (d * NDEVT + np.arange(NDEVT)).reshape(NCORES, MCORE)
        v0 = np.zeros((8 * B, ECOLS), np.float32)   # row 8k + b
        mdt = np.zeros((8, ECOLS), np.float32)      # row k
        bmt = np.zeros((8, ECOLS), np.float32)
        for k in range(NCORES):
            cols = col_of_t[k]
            ids = gids[k]
            upd = (~is_tm1[ids]) & (ids < NREAL)
            mvals = np.where(upd, DT / taup[ids], 0.0).astype(np.float32)
            mdt[k, cols] = mvals
            bmt[k, cols] = (mvals * biasp[ids]).astype(np.float32)
            for b in range(B):
                v0[8 * k + b, cols] = vc[b, ids]
        m = {
            "bidx": _wrap16(bpos), "idx3": _wrap16(col_of_t.astype(np.int16)),
        }
        ix_w = _wrap16(idx1)
        for j in range(16):
            m[f"ix{j}"] = np.ascontiguousarray(ix_w[:, j * 100:(j + 1) * 100])
        WQ = LCORE // 32
        for j in range(32):
            m[f"wm{j}"] = np.ascontiguousarray(wmc[:, j * WQ:(j + 1) * WQ])
        for b in range(B):
            vb = v0[np.arange(NCORES) * 8 + b]
            m[f"v0{2 * b}"] = np.ascontiguousarray(vb[:, 0:HC])
            m[f"v0{2 * b + 1}"] = np.ascontiguousarray(vb[:, HC:ECOLS])
        for j in range(4):
            m[f"fold{j}"] = np.ascontiguousarray(F[32 * j:32 * (j + 1), :])
        m["mdt0"] = np.ascontiguousarray(mdt[:, 0:HC])
        m["mdt1"] = np.ascontiguousarray(mdt[:, HC:ECOLS])
        m["bm0"] = np.ascontiguousarray(bmt[:, 0:HC])
        m["bm1"] = np.ascontiguousarray(bmt[:, HC:ECOLS])
        in_maps.append(m)
        meta.append(col_of_t)
    return in_maps, meta


def kernel(**inputs):
    steps = int(inputs["steps"])
    bias = np.asarray(inputs["bias"])
    with_bias = bool(np.any(bias != 0))
    in_maps, _meta = _prep(**inputs)
    if not with_bias:
        for m in in_maps:
            m.pop("bm0")
            m.pop("bm1")
    key = (steps, with_bias)
    if key not in _cache:
        _cache[key] = _build(steps, with_bias)
    nc = _cache[key]
    res = run_bass_kernel_spmd(nc, in_maps, list(range(NDEV)))
    out = np.zeros((B, NREAL), np.float32)
    for d in range(NDEV):
        sl = res.results[d]["vout"].astype(np.float32)
        lo = d * NDEVT
        hi = min(lo + NDEVT, NREAL)
        out[:, lo:hi] = sl[:, :hi - lo]
    return out
